# revision 3
# baseline (speedup 1.0000x reference)
"""Trainium2 Bass kernel for nn_DiffNet (2-layer LSTM encoder/decoder + FC head).

Sharding: tensor-parallel over the hidden/gate dimension across 8 NeuronCores
(core k owns hidden rows [k*128,(k+1)*128) of both LSTM layers; FC head
replicated).  Activations are [hidden_partitions, batch] so batch=256 is the
matmul moving dimension.

v2: the per-step AllGathers of h0 and h1 ride SEPARATE collectives (the
baseline merged them, serializing every matmul of the next step behind one
collective).  Instruction order is chosen so matmuls that do not depend on
the latest gather execute inside each collective's flight window:

  encode body s (L1 skewed one step behind L0):
     Z0_s   = W0x.x_s + Whh0.h0f_{s-1}     -> act0 -> gather0_s
     Z1_s-1 = Wih1.h0f_{s-1} (start, data long ready)
            + Whh1.h1f_{s-2} (stop)        -> act1 -> gather1_{s-1}
     The L0 recurrence cycle is whh0+act0+gather0 only; L1's cycle is
     whh1+act1+gather1 (wih1 rides earlier) — both fit the same period and
     the PE fills each collective window with the other layer's matmuls.

  decode body t:  Z0_t x/whh0 parts fill the gather1_{t-1} window; FC1 is
     the chain head after gather1; fc_b2 is folded into the decode L0 gate
     bias (b0d = b0 + West@fc_b2) so the West matmul consumes est without
     the +fc_b2 of the current step; Z1_t's whh1 part fills the gather0_t
     window.

Self-contained: hardcodes all shapes; host-side numpy only reshapes/slices.
"""

import os

import numpy as np

L = 2
H = 1024
XD = 192
YD = 64
IN = XD + YD  # 256
B = 256
PRE_LEN = int(os.environ.get("DIFFNET_PRE", "64"))
FWD_LEN = int(os.environ.get("DIFFNET_FWD", "48"))
NCORES = 8
SL = H // NCORES  # 128 hidden rows per core
KT_H = H // 128  # 8 K-tiles to contract over a full hidden vector
NGATE = 4
# ablation knob (timing experiments only — results are numerically wrong):
# 1 = skip collective_compute, read local bounce instead (same DMA geometry)
_NOCC = os.environ.get("DIFFNET_NOCC", "0") == "1"

_CACHE = {}


def _shard_host(inputs):
    """Build per-core input dicts (numpy only: slice / transpose / reshape)."""
    f32 = np.float32

    pre_x = np.asarray(inputs["pre_x"], f32)
    pre_y = np.asarray(inputs["pre_y"], f32)
    fwd_x = np.asarray(inputs["forward_x"], f32)

    # Encoder input, step-major, [t, p(128), kt(2), b] so the DMA is contiguous
    xy = np.concatenate([pre_x, pre_y], axis=2)  # (PRE, B, IN)
    xpre = (
        xy.transpose(0, 2, 1)  # (PRE, IN, B)
        .reshape(PRE_LEN, 2, 128, B)
        .transpose(0, 2, 1, 3)  # (PRE, 128, 2, B)
        .astype(np.float16)
    )
    # Decoder exogenous input: [t, in(192), b]
    xfwd = fwd_x.transpose(0, 2, 1).astype(np.float16)  # (FWD, 192, B)

    w_ih_0 = np.asarray(inputs["w_ih_0"], f32).reshape(NGATE, H, IN)
    w_hh_0 = np.asarray(inputs["w_hh_0"], f32).reshape(NGATE, H, H)
    w_ih_1 = np.asarray(inputs["w_ih_1"], f32).reshape(NGATE, H, H)
    w_hh_1 = np.asarray(inputs["w_hh_1"], f32).reshape(NGATE, H, H)
    b0 = (np.asarray(inputs["b_ih_0"], f32) + np.asarray(inputs["b_hh_0"], f32)).reshape(NGATE, H)
    b1 = (np.asarray(inputs["b_ih_1"], f32) + np.asarray(inputs["b_hh_1"], f32)).reshape(NGATE, H)
    fc_w1 = np.asarray(inputs["fc_w1"], f32)
    fc_b1 = np.asarray(inputs["fc_b1"], f32)
    fc_w2 = np.asarray(inputs["fc_w2"], f32)
    fc_b2 = np.asarray(inputs["fc_b2"], f32)

    def lhsT_hid(w, k):
        """(4, H, K) gate-major weight -> lhsT [128, KT, 4, 128] for core k."""
        sl = w[:, k * SL : (k + 1) * SL, :]  # (4, 128, K)
        kdim = sl.shape[2]
        kt = kdim // 128
        return (
            sl.transpose(2, 0, 1)  # (K, 4, 128)
            .reshape(kt, 128, NGATE, SL)
            .transpose(1, 0, 2, 3)  # (128, kt, 4, 128)
            .reshape(128, kt * NGATE * SL)
            .copy()
        )

    # FC head replicated on every core
    fcw1T = fc_w1.T.reshape(KT_H, 128, H).transpose(1, 0, 2).reshape(128, KT_H * H).copy()
    fcw2T = fc_w2.T.reshape(KT_H, 128, YD).transpose(1, 0, 2).reshape(128, KT_H * YD).copy()

    maps = []
    for k in range(NCORES):
        sl = slice(k * SL, (k + 1) * SL)
        w0xT = lhsT_hid(w_ih_0, k)  # (128, 2*4*128)
        west = w_ih_0[:, sl, XD:]  # (4, 128, 64)
        westT = west.transpose(2, 0, 1).reshape(YD, NGATE * SL).copy()  # (64, 512)
        whh0T = lhsT_hid(w_hh_0, k)  # (128, 8*4*128)
        wih1T = lhsT_hid(w_ih_1, k)
        whh1T = lhsT_hid(w_hh_1, k)
        # decode L0 gate bias with West @ fc_b2 folded in: the West matmul
        # then consumes est WITHOUT the current step's +fc_b2
        b0_dec = b0[:, sl] + np.einsum("gry,y->gr", west, fc_b2)  # (4, 128)
        m = {
            "xpre": xpre,
            "xfwd": xfwd,
            "w0xT": w0xT.astype(np.float16),
            "westT": westT.astype(np.float16),
            "whh0T": whh0T.astype(np.float16),
            "wih1T": wih1T.astype(np.float16),
            "whh1T": whh1T.astype(np.float16),
            "fcw1T": fcw1T.astype(np.float16),
            "fcw2T": fcw2T.astype(np.float16),
            "b0": b0[:, sl].T.copy(),  # (128, 4)
            "b0d": b0_dec.T.copy(),  # (128, 4)
            "b1": b1[:, sl].T.copy(),
            "fcb1": fc_b1.reshape(KT_H, 128).T.copy(),  # (128, 8): bias per M-tile
            "fcb2": fc_b2.reshape(YD, 1).copy(),
            "lastyT": pre_y[-1].T.copy(),  # (64, 256)
        }
        maps.append(m)
    return maps


def _build_program():
    import concourse.bass as bass
    import concourse.mybir as mybir
    import concourse.tile as tile
    from concourse import bacc

    dt = mybir.dt
    AF = mybir.ActivationFunctionType
    F32 = dt.float32
    FR = dt.float16  # matmul operand dtype (FWL stays on; ~8x bf16 precision)

    nc = bacc.Bacc("TRN2", target_bir_lowering=False, debug=False, num_devices=NCORES)

    # ---- external I/O ----
    t_xpre = nc.dram_tensor("xpre", [PRE_LEN, 128, 2, B], FR, kind="ExternalInput")
    t_xfwd = nc.dram_tensor("xfwd", [FWD_LEN, XD, B], FR, kind="ExternalInput")
    t_w0xT = nc.dram_tensor("w0xT", [128, 2 * NGATE * SL], FR, kind="ExternalInput")
    t_westT = nc.dram_tensor("westT", [YD, NGATE * SL], FR, kind="ExternalInput")
    t_whh0T = nc.dram_tensor("whh0T", [128, KT_H * NGATE * SL], FR, kind="ExternalInput")
    t_wih1T = nc.dram_tensor("wih1T", [128, KT_H * NGATE * SL], FR, kind="ExternalInput")
    t_whh1T = nc.dram_tensor("whh1T", [128, KT_H * NGATE * SL], FR, kind="ExternalInput")
    t_fcw1T = nc.dram_tensor("fcw1T", [128, KT_H * H], FR, kind="ExternalInput")
    t_fcw2T = nc.dram_tensor("fcw2T", [128, KT_H * YD], FR, kind="ExternalInput")
    t_b0 = nc.dram_tensor("b0", [128, NGATE], F32, kind="ExternalInput")
    t_b0d = nc.dram_tensor("b0d", [128, NGATE], F32, kind="ExternalInput")
    t_b1 = nc.dram_tensor("b1", [128, NGATE], F32, kind="ExternalInput")
    t_fcb1 = nc.dram_tensor("fcb1", [128, KT_H], F32, kind="ExternalInput")
    t_fcb2 = nc.dram_tensor("fcb2", [YD, 1], F32, kind="ExternalInput")
    t_lastyT = nc.dram_tensor("lastyT", [YD, B], F32, kind="ExternalInput")
    t_out = nc.dram_tensor("est_out", [FWD_LEN, YD, B], F32, kind="ExternalOutput")

    RG = [list(range(NCORES))]

    with tile.TileContext(nc) as tc:
        with (
            tc.tile_pool(name="const", bufs=1) as const,
            tc.tile_pool(name="xload", bufs=4) as xload,
            tc.tile_pool(name="state", bufs=4) as state,
            tc.tile_pool(name="gact", bufs=6) as gact,
            tc.tile_pool(name="hfull", bufs=4) as hfull,
            tc.tile_pool(name="psum", bufs=4, space="PSUM") as psum,
            tc.tile_pool(name="psfc", bufs=2, space="PSUM") as psfc,
            tc.tile_pool(name="dbounce", bufs=4, space="DRAM") as dbounce,
            tc.tile_pool(name="dshared", bufs=4, space="DRAM") as dshared,
        ):
            # ---- load constants ----
            w0xT = const.tile([128, 2, NGATE, SL], FR)
            nc.sync.dma_start(out=w0xT, in_=t_w0xT.ap().rearrange("p (k g m) -> p k g m", k=2, g=NGATE))
            westT = const.tile([YD, NGATE, SL], FR)
            nc.sync.dma_start(out=westT, in_=t_westT.ap().rearrange("p (g m) -> p g m", g=NGATE))
            whh0T = const.tile([128, KT_H, NGATE, SL], FR)
            nc.sync.dma_start(out=whh0T, in_=t_whh0T.ap().rearrange("p (k g m) -> p k g m", k=KT_H, g=NGATE))
            wih1T = const.tile([128, KT_H, NGATE, SL], FR)
            nc.sync.dma_start(out=wih1T, in_=t_wih1T.ap().rearrange("p (k g m) -> p k g m", k=KT_H, g=NGATE))
            whh1T = const.tile([128, KT_H, NGATE, SL], FR)
            nc.sync.dma_start(out=whh1T, in_=t_whh1T.ap().rearrange("p (k g m) -> p k g m", k=KT_H, g=NGATE))
            fcw1T = const.tile([128, KT_H, H], FR)
            nc.sync.dma_start(out=fcw1T, in_=t_fcw1T.ap().rearrange("p (k m) -> p k m", k=KT_H))
            fcw2T = const.tile([128, KT_H, YD], FR)
            nc.sync.dma_start(out=fcw2T, in_=t_fcw2T.ap().rearrange("p (k m) -> p k m", k=KT_H))
            b0 = const.tile([128, NGATE], F32)
            nc.sync.dma_start(out=b0, in_=t_b0.ap())
            b0d = const.tile([128, NGATE], F32)
            nc.sync.dma_start(out=b0d, in_=t_b0d.ap())
            b1 = const.tile([128, NGATE], F32)
            nc.sync.dma_start(out=b1, in_=t_b1.ap())
            fcb1 = const.tile([128, KT_H], F32)
            nc.sync.dma_start(out=fcb1, in_=t_fcb1.ap())
            fcb2 = const.tile([YD, 1], F32)
            nc.sync.dma_start(out=fcb2, in_=t_fcb2.ap())

            # ---- persistent state ----
            est = const.tile([YD, B], F32)  # replicated running estimate (f32)
            nc.sync.dma_start(out=est, in_=t_lastyT.ap())
            c0 = const.tile([128, B], F32)
            nc.vector.memset(c0, 0.0)
            c1 = const.tile([128, B], F32)
            nc.vector.memset(c1, 0.0)

            def z_tiles(name):
                """4 per-gate PSUM accumulators (one bank each; z0/z1 share
                the 4-buffer ring — z0 is always read out by act0 before z1's
                first matmul, and vice versa)."""
                ts = [psum.tile([128, B], F32, tag="z", name=f"{name}g{g}", bufs=5)
                      for g in range(NGATE)]
                return lambda g: ts[g]

            def lstm_act(zp, bias, cprev, tagp):
                """Gate activations + cell update. zp(g): PSUM slice [128,B].
                Returns (c_new, h_new(fp16))."""
                gi = gact.tile([128, B], F32, tag="gi", name="gi")
                gf = gact.tile([128, B], F32, tag="gf", name="gf")
                gg = gact.tile([128, B], F32, tag="gg", name="gg")
                go = gact.tile([128, B], F32, tag="go", name="go")
                nc.scalar.activation(gi, zp(0), AF.Sigmoid, bias=bias[:, 0:1])
                nc.scalar.activation(gf, zp(1), AF.Sigmoid, bias=bias[:, 1:2])
                nc.scalar.activation(go, zp(3), AF.Sigmoid, bias=bias[:, 3:4])
                nc.scalar.activation(gg, zp(2), AF.Tanh, bias=bias[:, 2:3])
                fc_ = gact.tile([128, B], F32, tag="fc_", name="fc_")
                nc.vector.tensor_mul(fc_, gf, cprev)
                ig = gact.tile([128, B], F32, tag="ig", name="ig")
                nc.vector.tensor_mul(ig, gi, gg)
                cnew = state.tile([128, B], F32, tag=tagp, name="cnew")
                nc.vector.tensor_add(cnew, fc_, ig)
                tc_ = gact.tile([128, B], F32, tag="tc_", name="tc_")
                nc.scalar.activation(tc_, cnew, AF.Tanh)
                hnew = state.tile([128, B], FR, tag=tagp + "h", name="hnew")
                nc.vector.tensor_mul(hnew, go, tc_)
                return cnew, hnew

            def gather(hk, tag):
                """AllGather hk [128,B] fp16 -> full tile [128, NCORES, B].
                Unload split in two so consumers of low k-tiles start early."""
                inb = dbounce.tile([128, B], FR, tag="agi" + tag, name="agi" + tag)
                nc.sync.dma_start(out=inb, in_=hk[:])
                full = hfull.tile([128, NCORES, B], FR, tag="hf" + tag,
                                  name="hf" + tag)
                if _NOCC:
                    src = inb[:]
                    src = bass.AP(tensor=src.tensor, offset=src.offset,
                                  ap=[src.ap[0], [0, NCORES]] + src.ap[1:])
                    nc.sync.dma_start(out=full[:, :, :], in_=src)
                    return full
                outb = dshared.tile([NCORES * 128, B], FR, tag="ago" + tag,
                                    name="ago" + tag, addr_space="Shared")
                nc.gpsimd.collective_compute(
                    "AllGather",
                    mybir.AluOpType.bypass,
                    replica_groups=RG,
                    ins=[inb[:].opt()],
                    outs=[outb[:].opt()],
                )
                src = outb[:].rearrange("(k p) b -> p k b", p=128)
                half = NCORES // 2
                nc.sync.dma_start(out=full[:, 0:half, :], in_=src[:, 0:half, :])
                nc.sync.dma_start(out=full[:, half:, :], in_=src[:, half:, :])
                return full

            h0f = None  # gathered h0 of the latest encode/decode step
            h0f_prev = None
            h1f = None  # gathered h1 (lags one step in encode)
            h1k = None

            def z1_step(h0f_in, h1f_in, first_l1):
                """Z1 for one step: wih1.h0f_in (start) + whh1.h1f_in (stop).
                first_l1: no whh1 part (h1_{-1}=0)."""
                z1 = z_tiles("z1")
                for g in range(NGATE):
                    for k in range(KT_H):
                        nc.tensor.matmul(z1(g), wih1T[:, k, g, :], h0f_in[:, k, :],
                                         start=(k == 0),
                                         stop=(first_l1 and k == KT_H - 1))
                if not first_l1:
                    for g in range(NGATE):
                        for k in range(KT_H):
                            nc.tensor.matmul(z1(g), whh1T[:, k, g, :], h1f_in[:, k, :],
                                             start=False, stop=(k == KT_H - 1))
                return z1

            # ================= encode (L1 skewed one body behind L0) ========
            for s in range(PRE_LEN):
                # ---- Z0_s: x part, then whh0.h0f_{s-1} ----
                z0 = z_tiles("z0")
                xt = xload.tile([128, 2, B], FR, tag="x", name="xt")
                nc.sync.dma_start(out=xt, in_=t_xpre.ap()[s])
                for g in range(NGATE):
                    nc.tensor.matmul(z0(g), w0xT[:, 0, g, :], xt[:, 0, :],
                                     start=True, stop=False)
                    nc.tensor.matmul(z0(g), w0xT[:, 1, g, :], xt[:, 1, :],
                                     start=False, stop=(s == 0))
                if s > 0:
                    for g in range(NGATE):
                        for k in range(KT_H):
                            nc.tensor.matmul(z0(g), whh0T[:, k, g, :], h0f[:, k, :],
                                             start=False, stop=(k == KT_H - 1))
                c0, h0k = lstm_act(z0, b0, c0, "c0")
                h0f_prev = h0f
                h0f = gather(h0k, "0")

                # ---- Z1_{s-1}: runs inside gather0_s's window ----
                if s >= 1:
                    z1 = z1_step(h0f_prev, h1f, first_l1=(s == 1))
                    c1, h1k = lstm_act(z1, b1, c1, "c1")
                    h1f = gather(h1k, "1")

            # ---- encode flush: Z1 of the last encode step ----
            z1 = z1_step(h0f, h1f, first_l1=False)
            c1, h1k = lstm_act(z1, b1, c1, "c1")
            h1f = gather(h1k, "1")

            # ================= decode =================
            for t in range(FWD_LEN):
                last = t == FWD_LEN - 1
                # ---- Z0_t x/whh0 parts: fill the gather1_{t-1} window ----
                if not last:
                    z0 = z_tiles("z0")
                    xt = xload.tile([128, 2, B], FR, tag="x", name="xt")
                    nc.scalar.dma_start(out=xt[:, 0, :], in_=t_xfwd.ap()[t, 0:128, :])
                    nc.scalar.dma_start(out=xt[0:64, 1, :], in_=t_xfwd.ap()[t, 128:XD, :])
                    for g in range(NGATE):
                        nc.tensor.matmul(z0(g), w0xT[:, 0, g, :], xt[:, 0, :],
                                         start=True, stop=False)
                        nc.tensor.matmul(z0(g), w0xT[0:64, 1, g, :], xt[0:64, 1, :],
                                         start=False, stop=False)
                        for k in range(KT_H):
                            nc.tensor.matmul(z0(g), whh0T[:, k, g, :], h0f[:, k, :],
                                             start=False, stop=False)

                # ---- FC head (replicated): chain head after gather1 ----
                u = gact.tile([128, KT_H, B], FR, tag="u", name="u")
                for m in range(KT_H):
                    up = psfc.tile([128, B], F32, tag="u", name="up", bufs=2)
                    for k in range(KT_H):
                        nc.tensor.matmul(up, fcw1T[:, k, m * 128:(m + 1) * 128],
                                         h1f[:, k, :],
                                         start=(k == 0), stop=(k == KT_H - 1))
                    nc.scalar.activation(u[:, m, :], up, AF.Tanh,
                                         bias=fcb1[:, m:m + 1])
                pp = psfc.tile([YD, B], F32, tag="p", name="pp", bufs=1)
                for k in range(KT_H):
                    nc.tensor.matmul(pp, fcw2T[:, k, :], u[:, k, :],
                                     start=(k == 0), stop=(k == KT_H - 1))
                # chain: est (fp16, no +fcb2 — folded into b0d) -> West matmul
                estr = state.tile([YD, B], FR, tag="estr", name="estr")
                nc.vector.tensor_add(estr, est, pp)
                # off-chain: full est (with fcb2) for output + next-step carry
                efull = state.tile([YD, B], F32, tag="est", name="efull")
                nc.vector.tensor_add(efull, est, pp)
                eout = state.tile([YD, B], F32, tag="esto", name="eout")
                nc.vector.tensor_scalar_add(eout, efull, fcb2[:, 0:1])
                nc.scalar.dma_start(out=t_out.ap()[t], in_=eout)
                est = eout
                if last:
                    break

                # close z0 with the West est K-tile
                for g in range(NGATE):
                    nc.tensor.matmul(z0(g), westT[:, g, :], estr,
                                     start=False, stop=True)
                c0, h0k = lstm_act(z0, b0d, c0, "c0")
                h0f = gather(h0k, "0")

                # ---- Z1_t: whh1 part (ready) fills gather0_t's window ----
                z1 = z_tiles("z1")
                for g in range(NGATE):
                    for k in range(KT_H):
                        nc.tensor.matmul(z1(g), whh1T[:, k, g, :], h1f[:, k, :],
                                         start=(k == 0), stop=False)
                for g in range(NGATE):
                    for k in range(KT_H):
                        nc.tensor.matmul(z1(g), wih1T[:, k, g, :], h0f[:, k, :],
                                         start=False, stop=(k == KT_H - 1))
                c1, h1k = lstm_act(z1, b1, c1, "c1")
                h1f = gather(h1k, "1")

    nc.compile()
    return nc


def kernel(**inputs) -> np.ndarray:
    from concourse.bass_utils import run_bass_kernel_spmd

    key = "prog"
    if key not in _CACHE:
        _CACHE[key] = _build_program()
    nc = _CACHE[key]

    in_maps = _shard_host(inputs)
    res = run_bass_kernel_spmd(nc, in_maps, core_ids=list(range(NCORES)))
    est = np.asarray(res.results[0]["est_out"])  # (FWD, YD, B)
    return est.transpose(0, 2, 1).astype(np.float32).copy()  # (FWD, B, YD)
